# revision 8
# baseline (speedup 1.0000x reference)
"""DeMBR multi-behavior LightGCN kernel for Trainium2 (8 NeuronCores).

v2 strategy (per dense behavior, each [N,N] relation matrix R):
  - Host pre-casts R to bf16. Row-shard across 8 cores (512 users each).
  - Each core loads its natural-layout shard ONCE from HBM (sync queue),
    then builds the item-major (transposed) copy At on-device with
    SBUF->SBUF DMA-xbar transposes (scalar queue). Halves HBM traffic.
  - PE passes per behavior (moving operand 512-wide):
      C2:    u1_un.T (+deg_u via ones column)  = [i0|1].T @ At
      C1+C4: zs.T | w.T packed                 = [u0|u0+u1].T @ Rn
             (zs = z * (1/deg_i) fused into the PSUM drain on DVE)
      C3:    u2_un.T = i1.T @ At               (after AllReduce)
  - One [64, 4096] bf16 AllReduce per behavior combines the pre-scaled
    zs = (R^T u0)/deg_i across cores; its result transposes straight
    into C3's stationary (no post-AR scaling).
  - Outputs are unnormalized transposes (u1T [65,512] incl. deg_u row,
    u2T [64,512]) plus w partials (wT bf16 [64,4096]); the host does the
    cheap O(N*D) normalize/combine and the 8-way w reduction.
  - Output DMAs + collectives ride the gpsimd queue so the two HWDGE
    queues (sync: Rn loads, scalar: xbar transposes) never stall.
  - A tiny warm-up AllReduce at t~0 absorbs the one-time cross-core
    rendezvous barrier concurrently with the first Rn load.
  - All-ones matrices (virtual behaviors at init) are detected on the
    host and computed analytically.

kernel(**inputs) takes the full unsharded inputs and returns [14, 4096, 64].
"""

import os
import numpy as np
import ml_dtypes

EPS = 1e-8
N, D = 4096, 64
P = 128
NCORES = 8
ULOC = N // NCORES          # 512 users per core
NU = ULOC // P              # 4 user chunks
NI = N // P                 # 32 item chunks
CH = 512                    # moving free-dim chunk
NCH = N // CH               # 8 chunks for the user-side contractions

_BF16 = ml_dtypes.bfloat16


# --------------------------------------------------------------------------
# device program
# --------------------------------------------------------------------------

def build_program(nb):
    """Build + bacc-compile the SPMD program for `nb` dense behaviors."""
    import concourse.bass as bass  # noqa: F401  (registers types)
    import concourse.mybir as mybir
    import concourse.tile as tile
    from concourse import bacc
    from concourse.masks import make_identity

    f32, bf16 = mybir.dt.float32, mybir.dt.bfloat16
    ALU = mybir.AluOpType

    nc = bacc.Bacc("TRN2", target_bir_lowering=False, debug=False,
                   num_devices=NCORES)

    R_in = [nc.dram_tensor(f"R{b}", [ULOC, N], bf16, kind="ExternalInput")
            for b in range(nb)]
    rib_in = [nc.dram_tensor(f"rib{b}", [D, N], f32, kind="ExternalInput")
              for b in range(nb)]
    i0s_in = nc.dram_tensor("i0s", [P, NI, D + 1], bf16, kind="ExternalInput")
    u0s_in = nc.dram_tensor("u0s", [P, NU, D], bf16, kind="ExternalInput")
    u1T_out = [nc.dram_tensor(f"u1T{b}", [D + 1, ULOC], f32,
                              kind="ExternalOutput") for b in range(nb)]
    u2T_out = [nc.dram_tensor(f"u2T{b}", [D, ULOC], f32,
                              kind="ExternalOutput") for b in range(nb)]
    w_out = [nc.dram_tensor(f"wT{b}", [D, N], bf16, kind="ExternalOutput")
             for b in range(nb)]

    rg = [list(range(NCORES))]

    with tile.TileContext(nc) as tc:
        with (
            tc.tile_pool(name="rn", bufs=2) as pRn,
            tc.tile_pool(name="at", bufs=2) as pAt,
            tc.tile_pool(name="rib", bufs=2) as pRib,
            tc.tile_pool(name="zw", bufs=2) as pzw,
            tc.tile_pool(name="small", bufs=2) as psm,
            tc.tile_pool(name="keep", bufs=1) as pkeep,
            tc.tile_pool(name="one", bufs=1) as pone,
            tc.tile_pool(name="mm", bufs=2, space="PSUM") as pmm,
            tc.tile_pool(name="mm14", bufs=2, space="PSUM") as pmm14,
            tc.tile_pool(name="tr", bufs=1, space="PSUM") as ptr,
            tc.tile_pool(name="dram", bufs=2, space="DRAM") as pdr,
        ):
            # ---- warm-up collective to absorb the startup rendezvous
            warm_in = pdr.tile([1, 64], bf16, tag="warm_in", name="warm_in")
            warm_out = pdr.tile([1, 64], bf16, tag="warm_out",
                                name="warm_out", addr_space="Shared")
            warm_sb = pone.tile([1, 64], bf16, tag="warm_sb", name="warm_sb")
            nc.vector.memset(warm_sb[:], 0.0)
            nc.gpsimd.dma_start(out=warm_in[:], in_=warm_sb[:])
            nc.gpsimd.collective_compute(
                "AllReduce", ALU.add, replica_groups=rg,
                ins=[warm_in.opt()], outs=[warm_out.opt()])

            ident = pone.tile([P, P], f32)
            make_identity(nc, ident[:])
            i0s = pone.tile([P, NI, D + 1], bf16)
            nc.scalar.dma_start(out=i0s[:], in_=i0s_in[:])
            u0s = pone.tile([P, NU, D], bf16)
            nc.scalar.dma_start(out=u0s[:], in_=u0s_in[:])

            state = {}
            out_tiles = []

            def front(b):
                # ---- natural-layout shard, loaded once (sync queue),
                #      per-uc so the xbar transposes can start early
                Rn = pRn.tile([P, NU, N], bf16, tag="Rn", name=f"Rn{b}")
                src = R_in[b].ap().rearrange("(uc p) n -> p uc n", p=P)
                for uc in range(NU):
                    nc.sync.dma_start(out=Rn[:, uc, :], in_=src[:, uc, :])

                # ---- on-device transpose: At[i%128, i//128, u] (scalar q)
                At = pAt.tile([P, NI, ULOC], bf16, tag="At", name=f"At{b}")
                for uc in range(NU):
                    nc.scalar.dma_start_transpose(
                        out=At[:, :, uc * P:(uc + 1) * P], in_=Rn[:, uc, :])
                rib = pRib.tile([D, N], f32, tag="rib", name=f"rib{b}",
                                bufs=1)
                nc.scalar.dma_start(out=rib[:], in_=rib_in[b].ap())

                # ---- C2: psum [65, 512] = [i0|1].T @ At  (accum over items)
                P2 = pmm.tile([D + 1, CH], f32, tag="PC", name=f"P2_{b}")
                for ic in range(NI):
                    nc.tensor.matmul(P2[:], i0s[:, ic, :], At[:, ic, :],
                                     start=(ic == 0), stop=(ic == NI - 1))
                S2 = psm.tile([D + 1, CH], f32, tag="S2", name=f"S2_{b}")
                nc.vector.tensor_copy(out=S2[:], in_=P2[:])
                PT2 = ptr.tile([P, NU, D + 1], f32, tag="PT2", name=f"PT2_{b}")
                for uc in range(NU):
                    nc.tensor.transpose(PT2[:, uc, :],
                                        S2[:, uc * P:(uc + 1) * P],
                                        ident[0:D + 1, 0:D + 1])
                rut = psm.tile([P, NU, 1], f32, tag="rut", name=f"rut{b}")
                nc.vector.tensor_scalar_add(out=rut[:], in0=PT2[:, :, D:D + 1],
                                            scalar1=EPS)
                ru = psm.tile([P, NU, 1], f32, tag="ru", name=f"ru{b}")
                nc.vector.reciprocal(out=ru[:], in_=rut[:])
                u1b = psm.tile([P, NU, D], bf16, tag="u1b", name=f"u1b{b}")
                for uc in range(NU):
                    nc.vector.tensor_scalar_mul(out=u1b[:, uc, :],
                                                in0=PT2[:, uc, 0:D],
                                                scalar1=ru[:, uc, :])
                L = psm.tile([P, NU, 2 * D], bf16, tag="L", name=f"L{b}")
                nc.vector.tensor_copy(out=L[:, :, 0:D], in_=u0s[:])
                nc.vector.tensor_add(out=L[:, :, D:2 * D], in0=u0s[:], in1=u1b[:])

                # ---- C1+C4 packed: [u0 | u0+u1].T @ Rn -> zs.T | w.T
                #      zs = z/deg_i fused into the PSUM drain
                zTs = pzw.tile([D, N], bf16, tag="zTs", name=f"zTs{b}",
                               bufs=1)
                wTs = pzw.tile([D, N], bf16, tag="wTs", name=f"wTs{b}",
                               bufs=1)
                for n in range(NCH):
                    P14 = pmm14.tile([P, CH], f32, tag="P14",
                                     name=f"P14_{b}_{n}")
                    for uc in range(NU):
                        nc.tensor.matmul(P14[:], L[:, uc, :],
                                         Rn[:, uc, n * CH:(n + 1) * CH],
                                         start=(uc == 0), stop=(uc == NU - 1))
                    sl = slice(n * CH, (n + 1) * CH)
                    nc.vector.tensor_mul(out=zTs[:, sl], in0=P14[0:D, :],
                                         in1=rib[:, sl])
                    nc.vector.tensor_copy(out=wTs[:, sl], in_=P14[D:2 * D, :])

                # ---- z out + AllReduce + w partial out (gpsimd queue)
                z_in = pdr.tile([D, N], bf16, tag="z_in", name=f"z_in{b}")
                nc.gpsimd.dma_start(out=z_in[:], in_=zTs[:])
                z_out = pdr.tile([D, N], bf16, tag="z_out",
                                 name=f"z_out{b}", addr_space="Shared")
                nc.gpsimd.collective_compute(
                    "AllReduce", ALU.add, replica_groups=rg,
                    ins=[z_in.opt()], outs=[z_out.opt()])
                nc.gpsimd.dma_start(out=w_out[b].ap(), in_=wTs[:])
                nc.gpsimd.dma_start(out=u1T_out[b].ap(), in_=S2[:])
                state[b] = (At, z_out)

            def back(b):
                At, z_out = state.pop(b)

                # ---- i1 (pre-scaled) straight from the AllReduce result
                zs = psm.tile([D, N], bf16, tag="zs", name=f"zs{b}")
                nc.gpsimd.dma_start(out=zs[:], in_=z_out[:])
                i1b = psm.tile([P, NI, D], bf16, tag="i1b", name=f"i1b{b}")
                nc.scalar.dma_start_transpose(out=i1b[:], in_=zs[:])

                # ---- C3: u2_un.T = i1.T @ At
                P3 = pmm.tile([D, CH], f32, tag="PC", name=f"P3_{b}")
                for ic in range(NI):
                    nc.tensor.matmul(P3[:], i1b[:, ic, :], At[:, ic, :],
                                     start=(ic == 0), stop=(ic == NI - 1))
                S3 = pkeep.tile([D, CH], f32, tag=f"S3_{b}", name=f"S3_{b}")
                nc.vector.tensor_copy(out=S3[:], in_=P3[:])
                out_tiles.append((u2T_out[b], S3))

            # F0 F1 B0 F2 B1 F3 B2 B3: backs interleave so the At pool
            # (bufs=2) recycles; all final output DMAs go last (gpsimd)
            fe = be = 0
            while be < nb:
                if fe < nb and fe - be < 2:
                    front(fe)
                    fe += 1
                else:
                    back(be)
                    be += 1
            for dst, tile_ in out_tiles:
                nc.gpsimd.dma_start(out=dst.ap(), in_=tile_[:])

    nc.compile()
    return nc


# --------------------------------------------------------------------------
# host-side helpers
# --------------------------------------------------------------------------

def _swz_items_aug(i0):
    """[4096, D]+ones -> [128, 32, D+1] with item = ic*128 + p."""
    aug = np.concatenate([i0.astype(_BF16), np.ones((N, 1), _BF16)], axis=1)
    return np.ascontiguousarray(
        aug.reshape(NI, P, D + 1).transpose(1, 0, 2))


def _swz_users(x):
    """[512, C] -> [128, 4, C] with user = uc*128 + p."""
    return np.ascontiguousarray(x.reshape(NU, P, x.shape[1]).transpose(1, 0, 2))


def host_prep_behavior(R):
    """Cast to bf16 + compute item-degree reciprocal broadcast [64, N]."""
    Rb = R.astype(_BF16)
    deg = R.sum(axis=0, dtype=np.float64)
    ri_vec = (1.0 / (deg + EPS)).astype(np.float32)
    rib = np.ascontiguousarray(np.broadcast_to(ri_vec[None, :], (D, N)),
                               dtype=np.float32)
    return Rb, rib, deg.astype(np.float32)


def prep_in_maps(dense_mats, u0, i0):
    """dense_mats: list of (R_bf16 [N,N], rib [64, N] f32)."""
    i0s = _swz_items_aug(i0)
    in_maps = []
    for k in range(NCORES):
        m = {"i0s": i0s,
             "u0s": _swz_users(u0[k * ULOC:(k + 1) * ULOC].astype(_BF16))}
        for b, (Rb, rib) in enumerate(dense_mats):
            m[f"R{b}"] = np.ascontiguousarray(Rb[k * ULOC:(k + 1) * ULOC, :])
            m[f"rib{b}"] = rib
        in_maps.append(m)
    return in_maps


def assemble_dense(results, degs, nb):
    """Per-behavior (u_acc [N,D], i_acc [N,D]) from per-core outputs."""
    out = []
    for b in range(nb):
        u_parts = []
        for k in range(NCORES):
            u1T = results[k][f"u1T{b}"]          # [65, 512] f32
            u2T = results[k][f"u2T{b}"]          # [64, 512] f32
            degu = u1T[D] + np.float32(EPS)      # [512]
            u = (u1T[0:D] + u2T) / degu[None, :] # [64, 512]
            u_parts.append(u.T)
        u_acc = np.concatenate(u_parts, axis=0) * np.float32(0.5)
        w = np.sum([results[k][f"wT{b}"].astype(np.float32)
                    for k in range(NCORES)], axis=0, dtype=np.float32)
        i_acc = (w * np.float32(0.5)
                 / (degs[b] + np.float32(EPS))[None, :]).T
        out.append((np.ascontiguousarray(u_acc, dtype=np.float32),
                    np.ascontiguousarray(i_acc, dtype=np.float32)))
    return out


def ones_behavior(u0, i0):
    """Analytic LightGCN-2-layer outputs when R is all-ones [N, N]."""
    s_i = i0.astype(np.float64).sum(axis=0)
    s_u = u0.astype(np.float64).sum(axis=0)
    d = N + EPS
    u_row = (s_i / d + s_u * N / (d * d)) * 0.5
    i_row = (s_u / d + s_i * N / (d * d)) * 0.5
    u = np.broadcast_to(u_row.astype(np.float32), (N, D)).copy()
    it = np.broadcast_to(i_row.astype(np.float32), (N, D)).copy()
    return u, it


# --------------------------------------------------------------------------
# cached device runner (compile once per behavior-count, run many)
# --------------------------------------------------------------------------

_RUNNERS = {}


class _Runner:
    def __init__(self, nb):
        self.nb = nb
        self.nc = build_program(nb)
        self._jitted = None
        self._meta = None

    def _prep_jit(self):
        import jax
        import numpy as _np
        from jax.sharding import Mesh, PartitionSpec
        from jax.experimental.shard_map import shard_map
        from concourse import bass2jax
        from concourse.bass2jax import _bass_exec_p, partition_id_tensor
        import concourse.mybir as mybir

        bass2jax.install_neuronx_cc_hook()
        nc = self.nc
        partition_name = (nc.partition_id_tensor.name
                          if nc.partition_id_tensor else None)
        in_names, out_names, out_avals, zero_shapes = [], [], [], []
        for alloc in nc.m.functions[0].allocations:
            if not isinstance(alloc, mybir.MemoryLocationSet):
                continue
            name = alloc.memorylocations[0].name
            if alloc.kind == "ExternalInput":
                if name != partition_name:
                    in_names.append(name)
            elif alloc.kind == "ExternalOutput":
                shape = tuple(alloc.tensor_shape)
                dtype = mybir.dt.np(alloc.dtype)
                out_names.append(name)
                out_avals.append(jax.core.ShapedArray(shape, dtype))
                zero_shapes.append((shape, dtype))
        n_params = len(in_names)
        full_in_names = list(in_names) + list(out_names)
        if partition_name is not None:
            full_in_names.append(partition_name)

        def _body(*args):
            operands = list(args)
            if partition_name is not None:
                operands.append(partition_id_tensor())
            outs = _bass_exec_p.bind(
                *operands,
                out_avals=tuple(out_avals),
                in_names=tuple(full_in_names),
                out_names=tuple(out_names),
                lowering_input_output_aliases=(),
                sim_require_finite=True,
                sim_require_nnan=True,
                nc=nc,
            )
            return tuple(outs)

        devices = jax.devices()[:NCORES]
        mesh = Mesh(_np.asarray(devices), ("core",))
        n_outs = len(out_names)
        in_specs = (PartitionSpec("core"),) * (n_params + n_outs)
        out_specs = (PartitionSpec("core"),) * n_outs
        donate = tuple(range(n_params, n_params + n_outs))
        self._jitted = jax.jit(
            shard_map(_body, mesh=mesh, in_specs=in_specs,
                      out_specs=out_specs, check_rep=False),
            donate_argnums=donate, keep_unused=True)
        self._meta = (in_names, out_names, out_avals, zero_shapes, n_params)

    def run(self, in_maps):
        if self._jitted is None:
            self._prep_jit()
        import numpy as _np
        in_names, out_names, out_avals, zero_shapes, n_params = self._meta
        concat_in = [
            _np.concatenate([_np.asarray(in_maps[c][nm]) for c in range(NCORES)],
                            axis=0)
            for nm in in_names]
        concat_zeros = [_np.zeros((NCORES * s[0], *s[1:]), dt)
                        for (s, dt) in zero_shapes]
        out_arrs = self._jitted(*concat_in, *concat_zeros)
        results = []
        for c in range(NCORES):
            results.append({
                nm: _np.asarray(out_arrs[i]).reshape(
                    NCORES, *out_avals[i].shape)[c]
                for i, nm in enumerate(out_names)})
        return results

    def run_traced(self, in_maps, tmpdir=None):
        """Run through run_bass_kernel_spmd with NTFF tracing (recompiles)."""
        _install_trace_shims()
        from concourse.bass_utils import run_bass_kernel_spmd
        return run_bass_kernel_spmd(self.nc, in_maps,
                                    core_ids=list(range(NCORES)),
                                    trace=True, tmpdir=tmpdir)


def _install_trace_shims():
    """This image's antenv lacks axon_hooks (the NTFF-hook registry) and has
    no artifact bucket; recreate the hook from the boot recipe and make
    artifact upload a local no-op."""
    import sys, types, importlib.util

    if "antenv.axon_hooks" not in sys.modules:
        mod = types.ModuleType("antenv.axon_hooks")
        mod._hook = None

        def set_axon_ntff_profile_hook(h):
            mod._hook = h

        def get_axon_ntff_profile_hook():
            return mod._hook

        mod.set_axon_ntff_profile_hook = set_axon_ntff_profile_hook
        mod.get_axon_ntff_profile_hook = get_axon_ntff_profile_hook
        import antenv
        sys.modules["antenv.axon_hooks"] = mod
        antenv.axon_hooks = mod

        spec = importlib.util.spec_from_file_location(
            "trn_boot_shim", "/root/.axon_site/trn_agent_boot/trn_boot.py")
        boot = importlib.util.module_from_spec(spec)
        spec.loader.exec_module(boot)
        hook = boot._ntff_profile_via_ctypes("/opt/axon/libaxon_pjrt.so")
        mod._hook = hook

    import concourse.bass_utils as bu
    if not getattr(bu.upload_artifacts, "_is_local_shim", False):
        def _local_upload(tmpdir):
            return tmpdir
        _local_upload._is_local_shim = True
        bu.upload_artifacts = _local_upload


def get_runner(nb):
    if nb not in _RUNNERS:
        _RUNNERS[nb] = _Runner(nb)
    return _RUNNERS[nb]


# --------------------------------------------------------------------------
# entry point
# --------------------------------------------------------------------------

def _is_ones(a):
    return a[0, 0] == 1.0 and bool(np.all(a == np.float32(1.0)))


def kernel(**inputs):
    inputs = {k: np.asarray(v) for k, v in inputs.items()}
    u0 = np.ascontiguousarray(inputs["user_embedding"], dtype=np.float32)
    i0 = np.ascontiguousarray(inputs["item_embedding"], dtype=np.float32)

    real_names = ["R_click", "R_fav", "R_cart", "R_buy"]
    virt_names = [("M_click", "add_click"), ("M_fav", "add_fav"),
                  ("M_cart", "add_cart")]
    mats = [np.asarray(inputs[n], dtype=np.float32) for n in real_names]
    mats += [np.asarray(inputs[m], dtype=np.float32) for m, _ in virt_names]

    dense_idx = [j for j, a in enumerate(mats) if not _is_ones(a)]
    per_behavior = [None] * 7

    if dense_idx:
        nb = len(dense_idx)
        runner = get_runner(nb)
        prepped = [host_prep_behavior(mats[j]) for j in dense_idx]
        in_maps = prep_in_maps([(p[0], p[1]) for p in prepped], u0, i0)
        results = runner.run(in_maps)
        dense = assemble_dense(results, [p[2] for p in prepped], nb)
        for pos, j in enumerate(dense_idx):
            per_behavior[j] = dense[pos]

    ones_cache = None
    for j, a in enumerate(mats):
        if per_behavior[j] is None:
            if ones_cache is None:
                ones_cache = ones_behavior(u0, i0)
            per_behavior[j] = ones_cache

    ur = [per_behavior[j][0] for j in range(4)]
    ir = [per_behavior[j][1] for j in range(4)]
    uv = [per_behavior[4 + j][0] + np.asarray(inputs[virt_names[j][1]],
                                              dtype=np.float32)
          for j in range(3)]
    iv = [per_behavior[4 + j][1] for j in range(3)]

    out = np.concatenate(
        [np.stack(ur), np.stack(ir), np.stack(uv), np.stack(iv)], axis=0)
    return np.ascontiguousarray(out, dtype=np.float32)


# revision 16
# speedup vs baseline: 1.5465x; 1.5465x over previous
"""DeMBR multi-behavior LightGCN kernel for Trainium2 (8 NeuronCores).

v2 strategy (per dense behavior, each [N,N] relation matrix R):
  - Host pre-casts R to bf16. Row-shard across 8 cores (512 users each).
  - Each core loads its natural-layout shard ONCE from HBM (sync queue),
    then builds the item-major (transposed) copy At on-device with
    SBUF->SBUF DMA-xbar transposes (scalar queue). Halves HBM traffic.
  - PE passes per behavior (moving operand 512-wide):
      C2:    u1_un.T (+deg_u via ones column)  = [i0|1].T @ At
      C1+C4: zs.T | w.T packed                 = [u0|u0+u1].T @ Rn
             (zs = z * (1/deg_i) fused into the PSUM drain on DVE)
      C3:    u2_un.T = i1.T @ At               (after AllReduce)
  - One [64, 4096] bf16 AllReduce per behavior combines the pre-scaled
    zs = (R^T u0)/deg_i across cores; its result transposes straight
    into C3's stationary (no post-AR scaling).
  - Outputs are unnormalized transposes (u1T [65,512] incl. deg_u row,
    u2T [64,512]) plus w partials (wT bf16 [64,4096]); the host does the
    cheap O(N*D) normalize/combine and the 8-way w reduction.
  - Output DMAs + collectives ride the gpsimd queue so the two HWDGE
    queues (sync: Rn loads, scalar: xbar transposes) never stall.
  - A tiny warm-up AllReduce at t~0 absorbs the one-time cross-core
    rendezvous barrier concurrently with the first Rn load.
  - All-ones matrices (virtual behaviors at init) are detected on the
    host and computed analytically.

kernel(**inputs) takes the full unsharded inputs and returns [14, 4096, 64].
"""

import os
import numpy as np
import ml_dtypes

EPS = 1e-8
N, D = 4096, 64
P = 128
NCORES = 8
ULOC = N // NCORES          # 512 users per core
NU = ULOC // P              # 4 user chunks
NI = N // P                 # 32 item chunks
CH = 512                    # moving free-dim chunk
NCH = N // CH               # 8 chunks for the user-side contractions

_BF16 = ml_dtypes.bfloat16


# --------------------------------------------------------------------------
# device program
# --------------------------------------------------------------------------

def build_program(nb):
    """Build + bacc-compile the SPMD program for `nb` dense behaviors."""
    import concourse.bass as bass  # noqa: F401  (registers types)
    import concourse.mybir as mybir
    import concourse.tile as tile
    from concourse import bacc
    from concourse.masks import make_identity

    f32, bf16 = mybir.dt.float32, mybir.dt.bfloat16
    ALU = mybir.AluOpType

    nc = bacc.Bacc("TRN2", target_bir_lowering=False, debug=False,
                   num_devices=NCORES)

    R_in = [nc.dram_tensor(f"R{b}", [ULOC, N], bf16, kind="ExternalInput")
            for b in range(nb)]
    Rt_in = [nc.dram_tensor(f"Rt{b}", [N, ULOC], bf16, kind="ExternalInput")
             for b in range(nb)]
    rib_in = [nc.dram_tensor(f"rib{b}", [D, N], f32, kind="ExternalInput")
              for b in range(nb)]
    i0s_in = nc.dram_tensor("i0s", [P, NI, D + 1], bf16, kind="ExternalInput")
    u0s_in = nc.dram_tensor("u0s", [P, NU, D], bf16, kind="ExternalInput")
    u1T_out = [nc.dram_tensor(f"u1T{b}", [D + 1, ULOC], f32,
                              kind="ExternalOutput") for b in range(nb)]
    u2T_out = [nc.dram_tensor(f"u2T{b}", [D, ULOC], f32,
                              kind="ExternalOutput") for b in range(nb)]
    w_out = [nc.dram_tensor(f"wT{b}", [D, N], bf16, kind="ExternalOutput")
             for b in range(nb)]

    rg = [list(range(NCORES))]

    with tile.TileContext(nc) as tc:
        with (
            tc.tile_pool(name="at", bufs=4) as pAt,
            tc.tile_pool(name="chunk", bufs=4) as pchunk,
            tc.tile_pool(name="rib", bufs=1) as pRib,
            tc.tile_pool(name="zw", bufs=1) as pzw,
            tc.tile_pool(name="small", bufs=2) as psm,
            tc.tile_pool(name="keep", bufs=1) as pkeep,
            tc.tile_pool(name="one", bufs=1) as pone,
            tc.tile_pool(name="mm", bufs=2, space="PSUM") as pmm,
            tc.tile_pool(name="mm14", bufs=2, space="PSUM") as pmm14,
            tc.tile_pool(name="tr", bufs=1, space="PSUM") as ptr,
            tc.tile_pool(name="dram", bufs=2, space="DRAM") as pdr,
        ):
            # ---- warm-up collective to absorb the startup rendezvous
            warm_in = pdr.tile([1, 64], bf16, tag="warm_in", name="warm_in")
            warm_out = pdr.tile([1, 64], bf16, tag="warm_out",
                                name="warm_out", addr_space="Shared")
            warm_sb = pone.tile([1, 64], bf16, tag="warm_sb", name="warm_sb")
            nc.vector.memset(warm_sb[:], 0.0)
            nc.gpsimd.dma_start(out=warm_in[:], in_=warm_sb[:])
            nc.gpsimd.collective_compute(
                "AllReduce", ALU.add, replica_groups=rg,
                ins=[warm_in.opt()], outs=[warm_out.opt()])

            ident = pone.tile([P, P], f32)
            make_identity(nc, ident[:])
            i0s = pone.tile([P, NI, D + 1], bf16)
            nc.scalar.dma_start(out=i0s[:], in_=i0s_in[:])
            u0s = pone.tile([P, NU, D], bf16)
            nc.scalar.dma_start(out=u0s[:], in_=u0s_in[:])

            state = {}
            out_tiles = []

            def front(b):
                # ---- At: host-pretransposed shard, plain strided load in
                #      4 groups on the scalar HWDGE queue
                At = pAt.tile([P, NI, ULOC], bf16, tag="At", name=f"At{b}")
                q = NI // 4
                tsrc = Rt_in[b].ap().rearrange("(ic p) u -> p ic u", p=P)
                for g in range(4):
                    nc.scalar.dma_start(out=At[:, g * q:(g + 1) * q, :],
                                        in_=tsrc[:, g * q:(g + 1) * q, :])
                rib = pRib.tile([D, N], f32, tag="rib", name=f"rib{b}")
                nc.scalar.dma_start(out=rib[:], in_=rib_in[b].ap())

                # ---- C2: psum [65, 512] = [i0|1].T @ At  (accum over items)
                P2 = pmm.tile([D + 1, CH], f32, tag="PC", name=f"P2_{b}")
                for ic in range(NI):
                    nc.tensor.matmul(P2[:], i0s[:, ic, :], At[:, ic, :],
                                     start=(ic == 0), stop=(ic == NI - 1))
                S2 = psm.tile([D + 1, CH], f32, tag="S2", name=f"S2_{b}")
                nc.vector.tensor_copy(out=S2[:], in_=P2[:])
                PT2 = ptr.tile([P, NU, D + 1], f32, tag="PT2", name=f"PT2_{b}")
                for uc in range(NU):
                    nc.tensor.transpose(PT2[:, uc, :],
                                        S2[:, uc * P:(uc + 1) * P],
                                        ident[0:D + 1, 0:D + 1])
                rut = psm.tile([P, NU, 1], f32, tag="rut", name=f"rut{b}")
                nc.vector.tensor_scalar_add(out=rut[:], in0=PT2[:, :, D:D + 1],
                                            scalar1=EPS)
                ru = psm.tile([P, NU, 1], f32, tag="ru", name=f"ru{b}")
                nc.vector.reciprocal(out=ru[:], in_=rut[:])
                u1b = psm.tile([P, NU, D], bf16, tag="u1b", name=f"u1b{b}",
                               bufs=1)
                for uc in range(NU):
                    nc.vector.tensor_scalar_mul(out=u1b[:, uc, :],
                                                in0=PT2[:, uc, 0:D],
                                                scalar1=ru[:, uc, :])
                L = psm.tile([P, NU, 2 * D], bf16, tag="L", name=f"L{b}")
                nc.vector.tensor_copy(out=L[:, :, 0:D], in_=u0s[:])
                nc.vector.tensor_add(out=L[:, :, D:2 * D], in0=u0s[:], in1=u1b[:])

                # ---- C1+C4 packed: [u0 | u0+u1].T @ R -> zs.T | w.T
                #      R streamed from DRAM in [128, 4, 512] chunks (sync q);
                #      zs = z/deg_i fused into the PSUM drain
                zTs = pzw.tile([D, N], bf16, tag="zTs", name=f"zTs{b}")
                wTs = pzw.tile([D, N], bf16, tag="wTs", name=f"wTs{b}")
                nsrc = R_in[b].ap().rearrange("(uc p) n -> p uc n", p=P)
                for n in range(NCH):
                    Ac = pchunk.tile([P, NU, CH], bf16, tag="Ac",
                                     name=f"Ac{b}_{n}")
                    nc.sync.dma_start(out=Ac[:],
                                      in_=nsrc[:, :, n * CH:(n + 1) * CH])
                    P14 = pmm14.tile([P, CH], f32, tag="P14",
                                     name=f"P14_{b}_{n}")
                    for uc in range(NU):
                        nc.tensor.matmul(P14[:], L[:, uc, :], Ac[:, uc, :],
                                         start=(uc == 0), stop=(uc == NU - 1))
                    sl = slice(n * CH, (n + 1) * CH)
                    nc.vector.tensor_mul(out=zTs[:, sl], in0=P14[0:D, :],
                                         in1=rib[:, sl])
                    nc.vector.tensor_copy(out=wTs[:, sl], in_=P14[D:2 * D, :])

                # ---- z out + AllReduce + w partial out (gpsimd queue)
                z_in = pdr.tile([D, N], bf16, tag="z_in", name=f"z_in{b}")
                nc.gpsimd.dma_start(out=z_in[:], in_=zTs[:])
                z_out = pdr.tile([D, N], bf16, tag="z_out",
                                 name=f"z_out{b}", addr_space="Shared")
                nc.gpsimd.collective_compute(
                    "AllReduce", ALU.add, replica_groups=rg,
                    ins=[z_in.opt()], outs=[z_out.opt()])
                nc.gpsimd.dma_start(out=w_out[b].ap(), in_=wTs[:])
                nc.gpsimd.dma_start(out=u1T_out[b].ap(), in_=S2[:])
                state[b] = (At, z_out)

            def back(b):
                At, z_out = state.pop(b)

                # ---- i1 (pre-scaled) straight from the AllReduce result
                zs = psm.tile([D, N], bf16, tag="zs", name=f"zs{b}", bufs=1)
                nc.gpsimd.dma_start(out=zs[:], in_=z_out[:])
                i1b = psm.tile([P, NI, D], bf16, tag="i1b", name=f"i1b{b}",
                               bufs=1)
                nc.scalar.dma_start_transpose(out=i1b[:], in_=zs[:])

                # ---- C3: u2_un.T = i1.T @ At
                P3 = pmm.tile([D, CH], f32, tag="PC", name=f"P3_{b}")
                for ic in range(NI):
                    nc.tensor.matmul(P3[:], i1b[:, ic, :], At[:, ic, :],
                                     start=(ic == 0), stop=(ic == NI - 1))
                S3 = pkeep.tile([D, CH], f32, tag=f"S3_{b}", name=f"S3_{b}")
                nc.vector.tensor_copy(out=S3[:], in_=P3[:])
                out_tiles.append((u2T_out[b], S3))

            # all fronts first (ARs fire early and evenly), then backs;
            # final output DMAs go last (gpsimd)
            for b in range(nb):
                front(b)
            for b in range(nb):
                back(b)
            for dst, tile_ in out_tiles:
                nc.gpsimd.dma_start(out=dst.ap(), in_=tile_[:])

    nc.compile()
    return nc


# --------------------------------------------------------------------------
# host-side helpers
# --------------------------------------------------------------------------

def _swz_items_aug(i0):
    """[4096, D]+ones -> [128, 32, D+1] with item = ic*128 + p."""
    aug = np.concatenate([i0.astype(_BF16), np.ones((N, 1), _BF16)], axis=1)
    return np.ascontiguousarray(
        aug.reshape(NI, P, D + 1).transpose(1, 0, 2))


def _swz_users(x):
    """[512, C] -> [128, 4, C] with user = uc*128 + p."""
    return np.ascontiguousarray(x.reshape(NU, P, x.shape[1]).transpose(1, 0, 2))


def host_prep_behavior(R):
    """Cast to bf16 + compute item-degree reciprocal broadcast [64, N]."""
    Rb = R.astype(_BF16)
    deg = R.sum(axis=0, dtype=np.float64)
    ri_vec = (1.0 / (deg + EPS)).astype(np.float32)
    rib = np.ascontiguousarray(np.broadcast_to(ri_vec[None, :], (D, N)),
                               dtype=np.float32)
    return Rb, rib, deg.astype(np.float32)


def prep_in_maps(dense_mats, u0, i0):
    """dense_mats: list of (R_bf16 [N,N], rib [64, N] f32)."""
    i0s = _swz_items_aug(i0)
    in_maps = []
    for k in range(NCORES):
        m = {"i0s": i0s,
             "u0s": _swz_users(u0[k * ULOC:(k + 1) * ULOC].astype(_BF16))}
        for b, (Rb, rib) in enumerate(dense_mats):
            shard = Rb[k * ULOC:(k + 1) * ULOC, :]
            m[f"R{b}"] = np.ascontiguousarray(shard)
            m[f"Rt{b}"] = np.ascontiguousarray(shard.T)
            m[f"rib{b}"] = rib
        in_maps.append(m)
    return in_maps


def assemble_dense(results, degs, nb):
    """Per-behavior (u_acc [N,D], i_acc [N,D]) from per-core outputs."""
    out = []
    for b in range(nb):
        u_parts = []
        for k in range(NCORES):
            u1T = results[k][f"u1T{b}"]          # [65, 512] f32
            u2T = results[k][f"u2T{b}"]          # [64, 512] f32
            degu = u1T[D] + np.float32(EPS)      # [512]
            u = (u1T[0:D] + u2T) / degu[None, :] # [64, 512]
            u_parts.append(u.T)
        u_acc = np.concatenate(u_parts, axis=0) * np.float32(0.5)
        w = np.sum([results[k][f"wT{b}"].astype(np.float32)
                    for k in range(NCORES)], axis=0, dtype=np.float32)
        i_acc = (w * np.float32(0.5)
                 / (degs[b] + np.float32(EPS))[None, :]).T
        out.append((np.ascontiguousarray(u_acc, dtype=np.float32),
                    np.ascontiguousarray(i_acc, dtype=np.float32)))
    return out


def ones_behavior(u0, i0):
    """Analytic LightGCN-2-layer outputs when R is all-ones [N, N]."""
    s_i = i0.astype(np.float64).sum(axis=0)
    s_u = u0.astype(np.float64).sum(axis=0)
    d = N + EPS
    u_row = (s_i / d + s_u * N / (d * d)) * 0.5
    i_row = (s_u / d + s_i * N / (d * d)) * 0.5
    u = np.broadcast_to(u_row.astype(np.float32), (N, D)).copy()
    it = np.broadcast_to(i_row.astype(np.float32), (N, D)).copy()
    return u, it


# --------------------------------------------------------------------------
# cached device runner (compile once per behavior-count, run many)
# --------------------------------------------------------------------------

_RUNNERS = {}


class _Runner:
    def __init__(self, nb):
        self.nb = nb
        self.nc = build_program(nb)
        self._jitted = None
        self._meta = None

    def _prep_jit(self):
        import jax
        import numpy as _np
        from jax.sharding import Mesh, PartitionSpec
        from jax.experimental.shard_map import shard_map
        from concourse import bass2jax
        from concourse.bass2jax import _bass_exec_p, partition_id_tensor
        import concourse.mybir as mybir

        bass2jax.install_neuronx_cc_hook()
        nc = self.nc
        partition_name = (nc.partition_id_tensor.name
                          if nc.partition_id_tensor else None)
        in_names, out_names, out_avals, zero_shapes = [], [], [], []
        for alloc in nc.m.functions[0].allocations:
            if not isinstance(alloc, mybir.MemoryLocationSet):
                continue
            name = alloc.memorylocations[0].name
            if alloc.kind == "ExternalInput":
                if name != partition_name:
                    in_names.append(name)
            elif alloc.kind == "ExternalOutput":
                shape = tuple(alloc.tensor_shape)
                dtype = mybir.dt.np(alloc.dtype)
                out_names.append(name)
                out_avals.append(jax.core.ShapedArray(shape, dtype))
                zero_shapes.append((shape, dtype))
        n_params = len(in_names)
        full_in_names = list(in_names) + list(out_names)
        if partition_name is not None:
            full_in_names.append(partition_name)

        def _body(*args):
            operands = list(args)
            if partition_name is not None:
                operands.append(partition_id_tensor())
            outs = _bass_exec_p.bind(
                *operands,
                out_avals=tuple(out_avals),
                in_names=tuple(full_in_names),
                out_names=tuple(out_names),
                lowering_input_output_aliases=(),
                sim_require_finite=True,
                sim_require_nnan=True,
                nc=nc,
            )
            return tuple(outs)

        devices = jax.devices()[:NCORES]
        mesh = Mesh(_np.asarray(devices), ("core",))
        n_outs = len(out_names)
        in_specs = (PartitionSpec("core"),) * (n_params + n_outs)
        out_specs = (PartitionSpec("core"),) * n_outs
        donate = tuple(range(n_params, n_params + n_outs))
        self._jitted = jax.jit(
            shard_map(_body, mesh=mesh, in_specs=in_specs,
                      out_specs=out_specs, check_rep=False),
            donate_argnums=donate, keep_unused=True)
        self._meta = (in_names, out_names, out_avals, zero_shapes, n_params)

    def run(self, in_maps):
        if self._jitted is None:
            self._prep_jit()
        import numpy as _np
        in_names, out_names, out_avals, zero_shapes, n_params = self._meta
        concat_in = [
            _np.concatenate([_np.asarray(in_maps[c][nm]) for c in range(NCORES)],
                            axis=0)
            for nm in in_names]
        concat_zeros = [_np.zeros((NCORES * s[0], *s[1:]), dt)
                        for (s, dt) in zero_shapes]
        out_arrs = self._jitted(*concat_in, *concat_zeros)
        results = []
        for c in range(NCORES):
            results.append({
                nm: _np.asarray(out_arrs[i]).reshape(
                    NCORES, *out_avals[i].shape)[c]
                for i, nm in enumerate(out_names)})
        return results

    def run_traced(self, in_maps, tmpdir=None):
        """Run through run_bass_kernel_spmd with NTFF tracing (recompiles)."""
        _install_trace_shims()
        from concourse.bass_utils import run_bass_kernel_spmd
        return run_bass_kernel_spmd(self.nc, in_maps,
                                    core_ids=list(range(NCORES)),
                                    trace=True, tmpdir=tmpdir)


def _install_trace_shims():
    """This image's antenv lacks axon_hooks (the NTFF-hook registry) and has
    no artifact bucket; recreate the hook from the boot recipe and make
    artifact upload a local no-op."""
    import sys, types, importlib.util

    if "antenv.axon_hooks" not in sys.modules:
        mod = types.ModuleType("antenv.axon_hooks")
        mod._hook = None

        def set_axon_ntff_profile_hook(h):
            mod._hook = h

        def get_axon_ntff_profile_hook():
            return mod._hook

        mod.set_axon_ntff_profile_hook = set_axon_ntff_profile_hook
        mod.get_axon_ntff_profile_hook = get_axon_ntff_profile_hook
        import antenv
        sys.modules["antenv.axon_hooks"] = mod
        antenv.axon_hooks = mod

        spec = importlib.util.spec_from_file_location(
            "trn_boot_shim", "/root/.axon_site/trn_agent_boot/trn_boot.py")
        boot = importlib.util.module_from_spec(spec)
        spec.loader.exec_module(boot)
        hook = boot._ntff_profile_via_ctypes("/opt/axon/libaxon_pjrt.so")
        mod._hook = hook

    import concourse.bass_utils as bu
    if not getattr(bu.upload_artifacts, "_is_local_shim", False):
        def _local_upload(tmpdir):
            return tmpdir
        _local_upload._is_local_shim = True
        bu.upload_artifacts = _local_upload


def get_runner(nb):
    if nb not in _RUNNERS:
        _RUNNERS[nb] = _Runner(nb)
    return _RUNNERS[nb]


# --------------------------------------------------------------------------
# entry point
# --------------------------------------------------------------------------

def _is_ones(a):
    return a[0, 0] == 1.0 and bool(np.all(a == np.float32(1.0)))


def kernel(**inputs):
    inputs = {k: np.asarray(v) for k, v in inputs.items()}
    u0 = np.ascontiguousarray(inputs["user_embedding"], dtype=np.float32)
    i0 = np.ascontiguousarray(inputs["item_embedding"], dtype=np.float32)

    real_names = ["R_click", "R_fav", "R_cart", "R_buy"]
    virt_names = [("M_click", "add_click"), ("M_fav", "add_fav"),
                  ("M_cart", "add_cart")]
    mats = [np.asarray(inputs[n], dtype=np.float32) for n in real_names]
    mats += [np.asarray(inputs[m], dtype=np.float32) for m, _ in virt_names]

    dense_idx = [j for j, a in enumerate(mats) if not _is_ones(a)]
    per_behavior = [None] * 7

    if dense_idx:
        nb = len(dense_idx)
        runner = get_runner(nb)
        prepped = [host_prep_behavior(mats[j]) for j in dense_idx]
        in_maps = prep_in_maps([(p[0], p[1]) for p in prepped], u0, i0)
        results = runner.run(in_maps)
        dense = assemble_dense(results, [p[2] for p in prepped], nb)
        for pos, j in enumerate(dense_idx):
            per_behavior[j] = dense[pos]

    ones_cache = None
    for j, a in enumerate(mats):
        if per_behavior[j] is None:
            if ones_cache is None:
                ones_cache = ones_behavior(u0, i0)
            per_behavior[j] = ones_cache

    ur = [per_behavior[j][0] for j in range(4)]
    ir = [per_behavior[j][1] for j in range(4)]
    uv = [per_behavior[4 + j][0] + np.asarray(inputs[virt_names[j][1]],
                                              dtype=np.float32)
          for j in range(3)]
    iv = [per_behavior[4 + j][1] for j in range(3)]

    out = np.concatenate(
        [np.stack(ur), np.stack(ir), np.stack(uv), np.stack(iv)], axis=0)
    return np.ascontiguousarray(out, dtype=np.float32)


# revision 23
# speedup vs baseline: 1.9498x; 1.2608x over previous
"""DeMBR multi-behavior LightGCN kernel for Trainium2 (8 NeuronCores).

v2 strategy (per dense behavior, each [N,N] relation matrix R):
  - Host pre-casts R to bf16. Row-shard across 8 cores (512 users each).
  - Each core loads its natural-layout shard ONCE from HBM (sync queue),
    then builds the item-major (transposed) copy At on-device with
    SBUF->SBUF DMA-xbar transposes (scalar queue). Halves HBM traffic.
  - PE passes per behavior (moving operand 512-wide):
      C2:    u1_un.T (+deg_u via ones column)  = [i0|1].T @ At
      C1+C4: zs.T | w.T packed                 = [u0|u0+u1].T @ Rn
             (zs = z * (1/deg_i) fused into the PSUM drain on DVE)
      C3:    u2_un.T = i1.T @ At               (after AllReduce)
  - One [64, 4096] bf16 AllReduce per behavior combines the pre-scaled
    zs = (R^T u0)/deg_i across cores; its result transposes straight
    into C3's stationary (no post-AR scaling).
  - Outputs are unnormalized transposes (u1T [65,512] incl. deg_u row,
    u2T [64,512]) plus w partials (wT bf16 [64,4096]); the host does the
    cheap O(N*D) normalize/combine and the 8-way w reduction.
  - Output DMAs + collectives ride the gpsimd queue so the two HWDGE
    queues (sync: Rn loads, scalar: xbar transposes) never stall.
  - A tiny warm-up AllReduce at t~0 absorbs the one-time cross-core
    rendezvous barrier concurrently with the first Rn load.
  - All-ones matrices (virtual behaviors at init) are detected on the
    host and computed analytically.

kernel(**inputs) takes the full unsharded inputs and returns [14, 4096, 64].
"""

import os
import numpy as np
import ml_dtypes

EPS = 1e-8
N, D = 4096, 64
P = 128
NCORES = 8
ULOC = N // NCORES          # 512 users per core
NU = ULOC // P              # 4 user chunks
NI = N // P                 # 32 item chunks
CH = 512                    # moving free-dim chunk
NCH = N // CH               # 8 chunks for the user-side contractions

_BF16 = ml_dtypes.bfloat16
_F8 = ml_dtypes.float8_e4m3


# --------------------------------------------------------------------------
# device program
# --------------------------------------------------------------------------

def build_program(nb):
    """Build + bacc-compile the SPMD program for `nb` dense behaviors."""
    import concourse.bass as bass  # noqa: F401  (registers types)
    import concourse.mybir as mybir
    import concourse.tile as tile
    from concourse import bacc
    from concourse.masks import make_identity

    f32, bf16 = mybir.dt.float32, mybir.dt.bfloat16
    f8 = mybir.dt.float8e4
    DR = mybir.MatmulPerfMode.DoubleRow
    ALU = mybir.AluOpType

    nc = bacc.Bacc("TRN2", target_bir_lowering=False, debug=False,
                   num_devices=NCORES)

    R_in = [nc.dram_tensor(f"R{b}", [ULOC, N], f8, kind="ExternalInput")
            for b in range(nb)]
    Rt_in = [nc.dram_tensor(f"Rt{b}", [N, ULOC], f8, kind="ExternalInput")
             for b in range(nb)]
    rib_in = [nc.dram_tensor(f"rib{b}", [D, N], f32, kind="ExternalInput")
              for b in range(nb)]
    rud_in = [nc.dram_tensor(f"rud{b}", [P, NU, 1], f32, kind="ExternalInput")
              for b in range(nb)]
    i0s_in = nc.dram_tensor("i0s", [P, NI, D], f8, kind="ExternalInput")
    u0s_in = nc.dram_tensor("u0s", [P, NU, D], f8, kind="ExternalInput")
    u1T_out = [nc.dram_tensor(f"u1T{b}", [D, ULOC], f32,
                              kind="ExternalOutput") for b in range(nb)]
    u2T_out = [nc.dram_tensor(f"u2T{b}", [D, ULOC], f32,
                              kind="ExternalOutput") for b in range(nb)]
    w_out = [nc.dram_tensor(f"wT{b}", [D, N], bf16, kind="ExternalOutput")
             for b in range(nb)]

    rg = [list(range(NCORES))]

    with tile.TileContext(nc) as tc:
        with (
            tc.tile_pool(name="at", bufs=4) as pAt,
            tc.tile_pool(name="chunk", bufs=4) as pchunk,
            tc.tile_pool(name="rib", bufs=1) as pRib,
            tc.tile_pool(name="zw", bufs=1) as pzw,
            tc.tile_pool(name="small", bufs=2) as psm,
            tc.tile_pool(name="keep", bufs=1) as pkeep,
            tc.tile_pool(name="one", bufs=1) as pone,
            tc.tile_pool(name="mm", bufs=2, space="PSUM") as pmm,
            tc.tile_pool(name="mm14", bufs=2, space="PSUM") as pmm14,
            tc.tile_pool(name="tr", bufs=1, space="PSUM") as ptr,
            tc.tile_pool(name="dram", bufs=2, space="DRAM") as pdr,
        ):
            ident = pone.tile([P, P], f32)
            make_identity(nc, ident[:])
            i0s = pone.tile([P, NI, D], f8)
            nc.sync.dma_start(out=i0s[:], in_=i0s_in[:])
            u0s = pone.tile([P, NU, D], f8)
            nc.sync.dma_start(out=u0s[:], in_=u0s_in[:])

            state = {}
            out_tiles = []

            def front(b):
                # ---- At: host-pretransposed fp8 shard, plain strided load
                #      in 4 groups on the scalar HWDGE queue
                At = pAt.tile([P, NI, ULOC], f8, tag="At", name=f"At{b}")
                q = NI // 4
                tsrc = Rt_in[b].ap().rearrange("(ic p) u -> p ic u", p=P)
                for g in range(4):
                    nc.scalar.dma_start(out=At[:, g * q:(g + 1) * q, :],
                                        in_=tsrc[:, g * q:(g + 1) * q, :])
                rib = pRib.tile([D, N], f32, tag="rib", name=f"rib{b}")
                nc.scalar.dma_start(out=rib[:], in_=rib_in[b].ap())
                rud = pRib.tile([P, NU, 1], f32, tag="rud", name=f"rud{b}")
                nc.scalar.dma_start(out=rud[:], in_=rud_in[b].ap())

                # ---- C2: psum [64, 512] = i0.T @ At  (fp8 DoubleRow,
                #      two item-chunks per matmul)
                P2 = pmm.tile([D, CH], f32, tag="PC", name=f"P2_{b}")
                for ic in range(0, NI, 2):
                    nc.tensor.matmul(P2[:], i0s[:, ic:ic + 2, :],
                                     At[:, ic:ic + 2, :], perf_mode=DR,
                                     start=(ic == 0), stop=(ic == NI - 2))
                S2 = psm.tile([D, CH], f32, tag="S2", name=f"S2_{b}")
                nc.vector.tensor_copy(out=S2[:], in_=P2[:])
                PT2 = ptr.tile([P, NU, D], f32, tag="PT2", name=f"PT2_{b}")
                for uc in range(NU):
                    nc.tensor.transpose(PT2[:, uc, :],
                                        S2[:, uc * P:(uc + 1) * P],
                                        ident[0:D, 0:D])
                u1b = psm.tile([P, NU, D], f8, tag="u1b", name=f"u1b{b}",
                               bufs=1)
                for uc in range(NU):
                    nc.vector.tensor_scalar_mul(out=u1b[:, uc, :],
                                                in0=PT2[:, uc, :],
                                                scalar1=rud[:, uc, :])
                Lw = psm.tile([P, NU, D], f8, tag="Lw", name=f"Lw{b}")
                nc.vector.tensor_add(out=Lw[:], in0=u0s[:], in1=u1b[:])

                # ---- C1+C4: z.T = u0.T @ R and w.T = (u0+u1).T @ R
                #      (fp8 DR, two user-chunks per matmul; R streamed in
                #      [128, 4, 512] chunks on the sync queue);
                #      zs = z/deg_i fused into the PSUM drain
                zTs = pzw.tile([D, N], bf16, tag="zTs", name=f"zTs{b}")
                wTs = pzw.tile([D, N], bf16, tag="wTs", name=f"wTs{b}")
                nsrc = R_in[b].ap().rearrange("(uc p) n -> p uc n", p=P)
                for n in range(NCH):
                    Ac = pchunk.tile([P, NU, CH], f8, tag="Ac",
                                     name=f"Ac{b}_{n}")
                    nc.sync.dma_start(out=Ac[:],
                                      in_=nsrc[:, :, n * CH:(n + 1) * CH])
                    Pz = pmm14.tile([D, CH], f32, tag="Pz",
                                    name=f"Pz_{b}_{n}")
                    Pw = pmm14.tile([D, CH], f32, tag="Pw",
                                    name=f"Pw_{b}_{n}")
                    for uc in range(0, NU, 2):
                        nc.tensor.matmul(Pz[:], u0s[:, uc:uc + 2, :],
                                         Ac[:, uc:uc + 2, :], perf_mode=DR,
                                         start=(uc == 0), stop=(uc == NU - 2))
                    for uc in range(0, NU, 2):
                        nc.tensor.matmul(Pw[:], Lw[:, uc:uc + 2, :],
                                         Ac[:, uc:uc + 2, :], perf_mode=DR,
                                         start=(uc == 0), stop=(uc == NU - 2))
                    sl = slice(n * CH, (n + 1) * CH)
                    nc.vector.tensor_mul(out=zTs[:, sl], in0=Pz[:],
                                         in1=rib[:, sl])
                    nc.vector.tensor_copy(out=wTs[:, sl], in_=Pw[:])

                # ---- z out + AllReduce + w partial out (gpsimd queue)
                z_in = pdr.tile([D, N], bf16, tag="z_in", name=f"z_in{b}")
                nc.gpsimd.dma_start(out=z_in[:], in_=zTs[:])
                z_out = pdr.tile([D, N], bf16, tag="z_out",
                                 name=f"z_out{b}", addr_space="Shared")
                nc.gpsimd.collective_compute(
                    "AllReduce", ALU.add, replica_groups=rg,
                    ins=[z_in.opt()], outs=[z_out.opt()])
                nc.gpsimd.dma_start(out=w_out[b].ap(), in_=wTs[:])
                nc.gpsimd.dma_start(out=u1T_out[b].ap(), in_=S2[:])
                state[b] = (At, z_out)

            def back(b):
                At, z_out = state.pop(b)

                # ---- i1 (pre-scaled) straight from the AllReduce result
                zs = psm.tile([D, N], bf16, tag="zs", name=f"zs{b}", bufs=1)
                nc.gpsimd.dma_start(out=zs[:], in_=z_out[:])
                i1b = psm.tile([P, NI, D], bf16, tag="i1b", name=f"i1b{b}",
                               bufs=1)
                nc.scalar.dma_start_transpose(out=i1b[:], in_=zs[:])
                i1f = psm.tile([P, NI, D], f8, tag="i1f", name=f"i1f{b}",
                               bufs=1)
                nc.vector.tensor_copy(out=i1f[:], in_=i1b[:])

                # ---- C3: u2_un.T = i1.T @ At  (fp8 DoubleRow)
                P3 = pmm.tile([D, CH], f32, tag="PC", name=f"P3_{b}")
                for ic in range(0, NI, 2):
                    nc.tensor.matmul(P3[:], i1f[:, ic:ic + 2, :],
                                     At[:, ic:ic + 2, :], perf_mode=DR,
                                     start=(ic == 0), stop=(ic == NI - 2))
                S3 = pkeep.tile([D, CH], f32, tag=f"S3_{b}", name=f"S3_{b}")
                nc.vector.tensor_copy(out=S3[:], in_=P3[:])
                out_tiles.append((u2T_out[b], S3))

            # all fronts first (ARs fire early and evenly), then backs;
            # final output DMAs go last (gpsimd)
            for b in range(nb):
                front(b)
            for b in range(nb):
                back(b)
            for dst, tile_ in out_tiles:
                nc.gpsimd.dma_start(out=dst.ap(), in_=tile_[:])

    nc.compile()
    return nc


# --------------------------------------------------------------------------
# host-side helpers
# --------------------------------------------------------------------------

def _swz_items(x, dt):
    """[4096, C] -> [128, 32, C] with item = ic*128 + p."""
    return np.ascontiguousarray(
        x.astype(dt).reshape(NI, P, x.shape[1]).transpose(1, 0, 2))


def _swz_users(x, dt):
    """[512, C] -> [128, 4, C] with user = uc*128 + p."""
    return np.ascontiguousarray(
        x.astype(dt).reshape(NU, P, x.shape[1]).transpose(1, 0, 2))


def host_prep_behavior(R):
    """Cast to fp8 + degree reciprocals (deg_i broadcast, deg_u swizzled)."""
    Rb = R.astype(_F8)
    deg_i = R.sum(axis=0, dtype=np.float64)
    ri_vec = (1.0 / (deg_i + EPS)).astype(np.float32)
    rib = np.ascontiguousarray(np.broadcast_to(ri_vec[None, :], (D, N)),
                               dtype=np.float32)
    deg_u = R.sum(axis=1, dtype=np.float64)
    rud_full = (1.0 / (deg_u + EPS)).astype(np.float32)
    return Rb, rib, deg_i.astype(np.float32), rud_full


def prep_in_maps(dense_mats, u0, i0):
    """dense_mats: list of (R_f8 [N,N], rib [64, N] f32, rud_full [N] f32)."""
    i0s = _swz_items(i0, _F8)
    in_maps = []
    for k in range(NCORES):
        sl = slice(k * ULOC, (k + 1) * ULOC)
        m = {"i0s": i0s, "u0s": _swz_users(u0[sl], _F8)}
        for b, (Rb, rib, rud_full) in enumerate(dense_mats):
            shard = Rb[sl, :]
            m[f"R{b}"] = np.ascontiguousarray(shard)
            m[f"Rt{b}"] = np.ascontiguousarray(shard.T)
            m[f"rib{b}"] = rib
            m[f"rud{b}"] = _swz_users(rud_full[sl, None], np.float32)
        in_maps.append(m)
    return in_maps


def assemble_dense(results, prepped_idx, nb):
    """Per-behavior (u_acc [N,D], i_acc [N,D]) from per-core outputs.
    prepped_idx: list of (deg_i [N] f32, rud_full [N] f32) per behavior."""
    out = []
    for b in range(nb):
        deg_i, rud_full = prepped_idx[b]
        u_parts = []
        for k in range(NCORES):
            sl = slice(k * ULOC, (k + 1) * ULOC)
            u1T = results[k][f"u1T{b}"]          # [64, 512] f32, unnormalized
            u2T = results[k][f"u2T{b}"]          # [64, 512] f32, unnormalized
            u = (u1T + u2T) * rud_full[sl][None, :]
            u_parts.append(u.T)
        u_acc = np.concatenate(u_parts, axis=0) * np.float32(0.5)
        w = np.sum([results[k][f"wT{b}"].astype(np.float32)
                    for k in range(NCORES)], axis=0, dtype=np.float32)
        i_acc = (w * np.float32(0.5)
                 / (deg_i + np.float32(EPS))[None, :]).T
        out.append((np.ascontiguousarray(u_acc, dtype=np.float32),
                    np.ascontiguousarray(i_acc, dtype=np.float32)))
    return out


def ones_behavior(u0, i0):
    """Analytic LightGCN-2-layer outputs when R is all-ones [N, N]."""
    s_i = i0.astype(np.float64).sum(axis=0)
    s_u = u0.astype(np.float64).sum(axis=0)
    d = N + EPS
    u_row = (s_i / d + s_u * N / (d * d)) * 0.5
    i_row = (s_u / d + s_i * N / (d * d)) * 0.5
    u = np.broadcast_to(u_row.astype(np.float32), (N, D)).copy()
    it = np.broadcast_to(i_row.astype(np.float32), (N, D)).copy()
    return u, it


# --------------------------------------------------------------------------
# cached device runner (compile once per behavior-count, run many)
# --------------------------------------------------------------------------

_RUNNERS = {}


class _Runner:
    def __init__(self, nb):
        self.nb = nb
        self.nc = build_program(nb)
        self._jitted = None
        self._meta = None

    def _prep_jit(self):
        import jax
        import numpy as _np
        from jax.sharding import Mesh, PartitionSpec
        from jax.experimental.shard_map import shard_map
        from concourse import bass2jax
        from concourse.bass2jax import _bass_exec_p, partition_id_tensor
        import concourse.mybir as mybir

        bass2jax.install_neuronx_cc_hook()
        nc = self.nc
        partition_name = (nc.partition_id_tensor.name
                          if nc.partition_id_tensor else None)
        in_names, out_names, out_avals, zero_shapes = [], [], [], []
        for alloc in nc.m.functions[0].allocations:
            if not isinstance(alloc, mybir.MemoryLocationSet):
                continue
            name = alloc.memorylocations[0].name
            if alloc.kind == "ExternalInput":
                if name != partition_name:
                    in_names.append(name)
            elif alloc.kind == "ExternalOutput":
                shape = tuple(alloc.tensor_shape)
                dtype = mybir.dt.np(alloc.dtype)
                out_names.append(name)
                out_avals.append(jax.core.ShapedArray(shape, dtype))
                zero_shapes.append((shape, dtype))
        n_params = len(in_names)
        full_in_names = list(in_names) + list(out_names)
        if partition_name is not None:
            full_in_names.append(partition_name)

        def _body(*args):
            operands = list(args)
            if partition_name is not None:
                operands.append(partition_id_tensor())
            outs = _bass_exec_p.bind(
                *operands,
                out_avals=tuple(out_avals),
                in_names=tuple(full_in_names),
                out_names=tuple(out_names),
                lowering_input_output_aliases=(),
                sim_require_finite=True,
                sim_require_nnan=True,
                nc=nc,
            )
            return tuple(outs)

        devices = jax.devices()[:NCORES]
        mesh = Mesh(_np.asarray(devices), ("core",))
        n_outs = len(out_names)
        in_specs = (PartitionSpec("core"),) * (n_params + n_outs)
        out_specs = (PartitionSpec("core"),) * n_outs
        donate = tuple(range(n_params, n_params + n_outs))
        self._jitted = jax.jit(
            shard_map(_body, mesh=mesh, in_specs=in_specs,
                      out_specs=out_specs, check_rep=False),
            donate_argnums=donate, keep_unused=True)
        self._meta = (in_names, out_names, out_avals, zero_shapes, n_params)

    def run(self, in_maps):
        if self._jitted is None:
            self._prep_jit()
        import numpy as _np
        in_names, out_names, out_avals, zero_shapes, n_params = self._meta
        concat_in = [
            _np.concatenate([_np.asarray(in_maps[c][nm]) for c in range(NCORES)],
                            axis=0)
            for nm in in_names]
        concat_zeros = [_np.zeros((NCORES * s[0], *s[1:]), dt)
                        for (s, dt) in zero_shapes]
        out_arrs = self._jitted(*concat_in, *concat_zeros)
        results = []
        for c in range(NCORES):
            results.append({
                nm: _np.asarray(out_arrs[i]).reshape(
                    NCORES, *out_avals[i].shape)[c]
                for i, nm in enumerate(out_names)})
        return results

    def run_traced(self, in_maps, tmpdir=None):
        """Run through run_bass_kernel_spmd with NTFF tracing (recompiles)."""
        _install_trace_shims()
        from concourse.bass_utils import run_bass_kernel_spmd
        return run_bass_kernel_spmd(self.nc, in_maps,
                                    core_ids=list(range(NCORES)),
                                    trace=True, tmpdir=tmpdir)


def _install_trace_shims():
    """This image's antenv lacks axon_hooks (the NTFF-hook registry) and has
    no artifact bucket; recreate the hook from the boot recipe and make
    artifact upload a local no-op."""
    import sys, types, importlib.util

    if "antenv.axon_hooks" not in sys.modules:
        mod = types.ModuleType("antenv.axon_hooks")
        mod._hook = None

        def set_axon_ntff_profile_hook(h):
            mod._hook = h

        def get_axon_ntff_profile_hook():
            return mod._hook

        mod.set_axon_ntff_profile_hook = set_axon_ntff_profile_hook
        mod.get_axon_ntff_profile_hook = get_axon_ntff_profile_hook
        import antenv
        sys.modules["antenv.axon_hooks"] = mod
        antenv.axon_hooks = mod

        spec = importlib.util.spec_from_file_location(
            "trn_boot_shim", "/root/.axon_site/trn_agent_boot/trn_boot.py")
        boot = importlib.util.module_from_spec(spec)
        spec.loader.exec_module(boot)
        hook = boot._ntff_profile_via_ctypes("/opt/axon/libaxon_pjrt.so")
        mod._hook = hook

    import concourse.bass_utils as bu
    if not getattr(bu.upload_artifacts, "_is_local_shim", False):
        def _local_upload(tmpdir):
            return tmpdir
        _local_upload._is_local_shim = True
        bu.upload_artifacts = _local_upload


def get_runner(nb):
    if nb not in _RUNNERS:
        _RUNNERS[nb] = _Runner(nb)
    return _RUNNERS[nb]


# --------------------------------------------------------------------------
# entry point
# --------------------------------------------------------------------------

def _is_ones(a):
    return a[0, 0] == 1.0 and bool(np.all(a == np.float32(1.0)))


def kernel(**inputs):
    inputs = {k: np.asarray(v) for k, v in inputs.items()}
    u0 = np.ascontiguousarray(inputs["user_embedding"], dtype=np.float32)
    i0 = np.ascontiguousarray(inputs["item_embedding"], dtype=np.float32)

    real_names = ["R_click", "R_fav", "R_cart", "R_buy"]
    virt_names = [("M_click", "add_click"), ("M_fav", "add_fav"),
                  ("M_cart", "add_cart")]
    mats = [np.asarray(inputs[n], dtype=np.float32) for n in real_names]
    mats += [np.asarray(inputs[m], dtype=np.float32) for m, _ in virt_names]

    dense_idx = [j for j, a in enumerate(mats) if not _is_ones(a)]
    per_behavior = [None] * 7

    if dense_idx:
        nb = len(dense_idx)
        runner = get_runner(nb)
        prepped = [host_prep_behavior(mats[j]) for j in dense_idx]
        in_maps = prep_in_maps([(p[0], p[1], p[3]) for p in prepped], u0, i0)
        results = runner.run(in_maps)
        dense = assemble_dense(results, [(p[2], p[3]) for p in prepped], nb)
        for pos, j in enumerate(dense_idx):
            per_behavior[j] = dense[pos]

    ones_cache = None
    for j, a in enumerate(mats):
        if per_behavior[j] is None:
            if ones_cache is None:
                ones_cache = ones_behavior(u0, i0)
            per_behavior[j] = ones_cache

    ur = [per_behavior[j][0] for j in range(4)]
    ir = [per_behavior[j][1] for j in range(4)]
    uv = [per_behavior[4 + j][0] + np.asarray(inputs[virt_names[j][1]],
                                              dtype=np.float32)
          for j in range(3)]
    iv = [per_behavior[4 + j][1] for j in range(3)]

    out = np.concatenate(
        [np.stack(ur), np.stack(ir), np.stack(uv), np.stack(iv)], axis=0)
    return np.ascontiguousarray(out, dtype=np.float32)
